# revision 24
# baseline (speedup 1.0000x reference)
"""Causal multi-head attention (B=2, S=2048, H=16, D=128, fp32) on 8 trn2 NeuronCores.

Sharding: the 32 (batch, head) pairs are split 4-per-core (head-parallel — the
endpoint of the Ulysses all-to-all; with full inputs on host, realized as the
host-side scatter/gather). Causal work per head is identical, so cores are
perfectly load-balanced and need no cross-core communication.

Device kernel (per core, per head): flash-style attention, S^T score layout,
P-stationary PV with a fused denominator column.
  - Host packs Q^T[d,s], K^T[d,s] and V_ext[s,130] (col 128 = 1.0 denominator
    column, col 129 pad) into one tensor per (head, 512-wide q/k-group):
    [qt 512 | kt 512 | v 4x130] = 1544 cols x 128 partitions, so each input
    group is ONE ~0.4MB DMA with 3KB partition lines (near-peak HBM BW).
  - QK: S^T[sk,sq] = KT_tile^T @ QT per 512-wide q-block, diagonal k-tiles
    trimmed and ragged-packed in pairs; full k-tiles batched 2 per PSUM tile.
  - exp via ACT (fused 1/sqrt(D) scale) for most tiles; late blocks offload
    some full k-tile groups to DVE via a Schraudolph int16/bf16 bit-trick exp
    so the exp cost splits across two engines. DVE also applies the
    triangular mask on the two diagonal 128-bands (strided mul).
  - PV: for each 128-row q-chunk, stationary = P^T chunk [sk,128sq], moving =
    V_ext[sk,130] -> PSUM chunk [128sq,130] accumulates [O | L | pad] over
    k-tiles: the softmax denominator L costs one extra moving column instead
    of ones-matmuls + DVE reduction trees.
  - DVE copies PSUM->SBUF bf16, one DMA out per block; host divides O by L
    during the gather (normalization off-device).
  - Blocks run in diagonal wavefront order (wave = head + j) with a
    one-block software pipeline; input DMAs are issued in need order.
"""

import math
import sys

sys.path.insert(0, "/opt/trn_rl_repo")

import numpy as np

B, S, H, D = 2, 2048, 16, 128
NCORES = 8
HPC = (B * H) // NCORES  # heads per core = 4
QB = 512                 # q-block width
NQB = S // QB            # 4
KT128 = S // 128         # 16 k-tiles per head
SCALE = 1.0 / math.sqrt(D)
GRP = 2                  # full k-tiles exp'd per ACT instruction
VW = 130                 # V width: 128 d + 1 ones (L) + 1 pad
PK = QB + QB + 4 * VW    # packed group width: qt 512 | kt 512 | v 4*130
# Schraudolph bf16 exp offload (DVE): exp(x*SCALE) ~ bitcast_bf16(int16(
# x*(2^7*log2(e)*SCALE) + (127*2^7 - C))). C centers the sawtooth error
# (rms ~1.8%); applied only to full k-tiles of late blocks where softmax
# rows average >= 512 terms, so the output error contribution stays ~1%.
SCH_A = (128.0 / math.log(2.0)) * (1.0 / math.sqrt(D))
SCH_B = 127.0 * 128.0 - 7.33

_COMPILED = {}
LAST_RESULT = None


def _build_bass():
    from contextlib import ExitStack

    import concourse.tile as tile
    from concourse import bacc, mybir

    f32 = mybir.dt.float32
    bf16 = mybir.dt.bfloat16
    Exp = mybir.ActivationFunctionType.Exp

    nc = bacc.Bacc(
        "TRN2",
        target_bir_lowering=False,
        debug=False,
        enable_asserts=False,
        num_devices=NCORES,
    )
    # packed input: [head, j-group, partition, qt|kt|v]
    p_d = nc.dram_tensor("pk", [HPC, NQB, 128, PK], bf16, kind="ExternalInput").ap()
    mk_d = nc.dram_tensor("mask", [128, 128], bf16, kind="ExternalInput").ap()
    # out[h, p, qi, :] = [O | L | pad] for query row qi*128 + p
    o_d = nc.dram_tensor("out", [HPC, 128, KT128, VW], bf16, kind="ExternalOutput").ap()

    with tile.TileContext(nc) as tc, ExitStack() as ctx:
        const = ctx.enter_context(tc.tile_pool(name="const", bufs=1))
        ptd0_pool = ctx.enter_context(tc.tile_pool(name="ptd0", bufs=3))
        ptd1_pool = ctx.enter_context(tc.tile_pool(name="ptd1", bufs=3))
        ptf_pool = ctx.enter_context(tc.tile_pool(name="ptf", bufs=8))
        osb_pool = ctx.enter_context(tc.tile_pool(name="osb", bufs=8))
        ps_s = ctx.enter_context(tc.tile_pool(name="ps_s", bufs=2, space="PSUM"))
        ps_v = ctx.enter_context(tc.tile_pool(name="ps_v", bufs=2, space="PSUM"))

        # per-head packed input tile: [128, j-group, qt|kt|v]
        pk = [
            const.tile([128, NQB, PK], bf16, name=f"pk{h}", tag=f"pk{h}")
            for h in range(HPC)
        ]
        mk_sb = const.tile([128, 128], bf16)

        def load(hh, g0, g1, eng):
            eng.dma_start(
                pk[hh][:, g0 : g1 + 1, :],
                p_d[hh][g0 : g1 + 1].rearrange("n p c -> p n c"),
            )

        # Input DMA issue in need order (wave = head + j). Head 0's first
        # group is split so the first matmul (diagonal pair m2/m3, which only
        # needs the [256:512] halves) can start as early as possible. Late
        # heads' groups are merged into single ~1.2MB DMAs for peak BW.
        nc.sync.dma_start(pk[0][:, 0, 256:QB], p_d[0][0][:, 256:QB])
        nc.scalar.dma_start(pk[0][:, 0, QB + 256 : 2 * QB], p_d[0][0][:, QB + 256 : 2 * QB])
        nc.sync.dma_start(pk[0][:, 0, 0:256], p_d[0][0][:, 0:256])
        nc.scalar.dma_start(pk[0][:, 0, QB : QB + 256], p_d[0][0][:, QB : QB + 256])
        nc.scalar.dma_start(mk_sb[:], mk_d[:])
        nc.sync.dma_start(pk[0][:, 0, 2 * QB : PK], p_d[0][0][:, 2 * QB : PK])
        loads = [
            (1, 0, 0),  # (hh, g0, g1) in need order
            (0, 1, 1),
            (1, 1, 1),
            (2, 0, 0),
            (0, 2, 2),
            (1, 2, 2),
            (2, 1, 1),
            (3, 0, 0),
            (0, 3, 3),
            (2, 2, 3),
            (1, 3, 3),
            (3, 1, 3),
        ]
        for n, (hh, g0, g1) in enumerate(loads):
            load(hh, g0, g1, nc.sync)

        def qt_ap(hh, j, w0):
            # columns [j*QB + w0, (j+1)*QB) of head hh's Q^T
            return pk[hh][:, j, w0:QB]

        def kt_ap(hh, ki):
            return pk[hh][:, ki // 4, QB + (ki % 4) * 128 : QB + (ki % 4 + 1) * 128]

        def v_ap(hh, ki):
            base = 2 * QB + (ki % 4) * VW
            return pk[hh][:, ki // 4, base : base + VW]

        # Diagonal k-tile m of a block: trimmed to width 512-128m, packed as
        # pair0 = {m0 @ 0 (512), m1 @ 512 (384)} in a [128,1024] tile and
        # pair1 = {m2 @ 0 (256), m3 @ 256 (128)} in a [128,512] tile.
        DIAG_OFF = {0: 0, 1: 512, 2: 0, 3: 256}

        def phase1(hh, j):
            """Allocate block tiles; return (state, generator). The generator
            emits QK matmuls + exp + mask one PSUM-group at a time, so the
            driver can interleave the previous block's PV chunks between
            groups (keeping PE fed while ACT drains the exp backlog)."""
            ptd0 = ptd0_pool.tile([128, 1024], bf16, tag="ptd0", name="ptd0")
            ptd1 = ptd1_pool.tile([128, 512], bf16, tag="ptd1", name="ptd1")
            nf = (4 * j + GRP - 1) // GRP
            ptfs = [
                ptf_pool.tile([128, GRP * QB], bf16, tag="ptf", name="ptf")
                for _ in range(nf)
            ]

            def gen():
                # j=0 emits pair1 (m2/m3) first: its inputs are the second
                # half of the first group, which lands first at startup.
                pair_order = (
                    ((1, ptd1), (0, ptd0)) if j == 0 else ((0, ptd0), (1, ptd1))
                )
                for pair, pt in pair_order:
                    ms = (0, 1) if pair == 0 else (2, 3)
                    tot = 896 if pair == 0 else 384
                    s_ps = ps_s.tile([128, GRP * QB], f32, tag="s", name="s_ps")
                    for m in ms:
                        w = QB - 128 * m
                        nc.tensor.matmul(
                            s_ps[:, DIAG_OFF[m] : DIAG_OFF[m] + w],
                            kt_ap(hh, 4 * j + m),
                            qt_ap(hh, j, 128 * m),
                            start=True,
                            stop=True,
                        )
                    nc.scalar.activation(
                        pt[:, 0:tot], s_ps[:, 0:tot], Exp, scale=SCALE
                    )
                    # Both 128-wide triangular bands of this pair sit at
                    # offsets {0, half}: mask with one strided DVE op.
                    half = 512 if pair == 0 else 256
                    ptv = pt[:, 0 : 2 * half].rearrange(
                        "p (a b) -> p a b", a=2, b=half
                    )[:, :, 0:128]
                    mkv = mk_sb[:].unsqueeze(1).broadcast_to([128, 2, 128])
                    nc.vector.tensor_mul(ptv, ptv, mkv)
                    yield
                # Full k-tiles, exp'd GRP at a time. Late blocks offload some
                # groups to DVE (Schraudolph) so exp splits across engines.
                for gi, g0 in enumerate(range(0, 4 * j, GRP)):
                    grp = min(GRP, 4 * j - g0)
                    s_ps = ps_s.tile([128, GRP * QB], f32, tag="s", name="s_ps")
                    ptf = ptfs[gi]
                    for idx in range(grp):
                        nc.tensor.matmul(
                            s_ps[:, idx * QB : (idx + 1) * QB],
                            kt_ap(hh, g0 + idx),
                            qt_ap(hh, j, 0),
                            start=True,
                            stop=True,
                        )
                    offload = (
                        (j == 1 and g0 == 2)
                        or (j == 2 and g0 == 6)
                        or (j == 3 and g0 >= 6)
                    )
                    if offload:
                        nc.vector.tensor_scalar(
                            ptf[:, 0 : grp * QB].bitcast(mybir.dt.int16),
                            s_ps[:, 0 : grp * QB],
                            SCH_A,
                            SCH_B,
                            mybir.AluOpType.mult,
                            mybir.AluOpType.add,
                        )
                    else:
                        nc.scalar.activation(
                            ptf[:, 0 : grp * QB],
                            s_ps[:, 0 : grp * QB],
                            Exp,
                            scale=SCALE,
                        )
                    yield

            return (hh, j, ptd0, ptd1, ptfs), gen()

        def pt_chunk(ptd0, ptd1, ptfs, j, ki, c):
            """Stationary AP: columns [c*128,(c+1)*128) of block j's P^T tile ki."""
            if ki < 4 * j:
                gi, idx = ki // GRP, ki % GRP
                base = idx * QB + c * 128
                return ptfs[gi][:, base : base + 128]
            m = ki - 4 * j
            pt = ptd0 if m < 2 else ptd1
            base = DIAG_OFF[m] + (c - m) * 128
            return pt[:, base : base + 128]

        def phase2(st):
            """Generator: PV accumulation [O | L | pad] one 128-row q-chunk
            per step, then copy PSUM->SBUF and DMA out."""
            hh, j, ptd0, ptd1, ptfs = st
            pv = [
                ps_v.tile([128, 2, VW], f32, tag="pv01", name="pv01"),
                ps_v.tile([128, 2, VW], f32, tag="pv23", name="pv23"),
            ]
            # Chunk-major: each PSUM accumulation group is contiguous (two
            # chunks share a bank; interleaving groups would corrupt the
            # bank-wide has_written bits).
            for c in range(4):
                dest = pv[c // 2][:, c % 2, :]
                last = 4 * j + c
                for ki in range(last + 1):
                    if ki >= 4 * j and (ki - 4 * j) > c:
                        continue
                    nc.tensor.matmul(
                        dest,
                        pt_chunk(ptd0, ptd1, ptfs, j, ki, c),
                        v_ap(hh, ki),
                        start=(ki == 0),
                        stop=(ki == last),
                    )
                yield
            osb = osb_pool.tile([128, 4, VW], bf16, tag="osb", name="osb")
            nc.vector.tensor_copy(osb[:, 0:2, :], pv[0][:])
            nc.vector.tensor_copy(osb[:, 2:4, :], pv[1][:])
            nc.sync.dma_start(o_d[hh][:, 4 * j : 4 * j + 4, :], osb[:])

        # One-block software pipeline with interleaved emission: the QK/exp
        # groups of block N+1 alternate with the PV chunks of block N in the
        # PE instruction stream, so whenever a QK group is gated on ACT
        # draining a PSUM tile, ready PV work sits right behind it.
        # Blocks run in diagonal wavefront order (wave s = head + j): early
        # waves touch few heads' inputs, so compute ramps while DMA streams
        # the rest, and consecutive blocks stay similar in shape.
        order = sorted(
            ((hh, j) for hh in range(HPC) for j in range(NQB)),
            key=lambda t: (t[0] + t[1], t[0]),
        )

        def drive(g1, g2):
            done1 = done2 = False
            while not (done1 and done2):
                if not done1:
                    done1 = next(g1, "end") == "end"
                if not done2:
                    done2 = next(g2, "end") == "end"

        prev = None
        for hh, j in order:
            st, g1 = phase1(hh, j)
            drive(g1, phase2(prev) if prev is not None else iter(()))
            prev = st
        drive(iter(()), phase2(prev))

    nc.compile()
    return nc


def _get_compiled():
    if "nc" not in _COMPILED:
        _COMPILED["nc"] = _build_bass()
    return _COMPILED["nc"]


def _make_mask():
    k = np.arange(128, dtype=np.int64)[:, None]
    t = np.arange(128, dtype=np.int64)[None, :]
    return (t >= k).astype(np.float32)


def kernel(query, key, value):
    global LAST_RESULT
    from concourse.bass_utils import run_bass_kernel_spmd

    import ml_dtypes

    bf16 = ml_dtypes.bfloat16

    # [B, S, H, D] -> [B*H, S, D]
    q = np.asarray(query, dtype=np.float32).transpose(0, 2, 1, 3).reshape(B * H, S, D)
    k = np.asarray(key, dtype=np.float32).transpose(0, 2, 1, 3).reshape(B * H, S, D)
    v = np.asarray(value, dtype=np.float32).transpose(0, 2, 1, 3).reshape(B * H, S, D)

    qt = q.transpose(0, 2, 1).astype(bf16)  # [BH, D, S]
    kt = k.transpose(0, 2, 1).astype(bf16)

    v_ext = np.zeros((B * H, S, VW), dtype=bf16)
    v_ext[:, :, :D] = v.astype(bf16)
    v_ext[:, :, D] = bf16(1.0)
    # v packed per j-group: [BH, NQB, 128, 4*VW], row (n*128+p) -> [p, n*VW+d]
    vp = (
        v_ext.reshape(B * H, NQB, 4, 128, VW)
        .transpose(0, 1, 3, 2, 4)
        .reshape(B * H, NQB, 128, 4 * VW)
    )

    pack = np.empty((B * H, NQB, 128, PK), dtype=bf16)
    for g in range(NQB):
        pack[:, g, :, 0:QB] = qt[:, :, g * QB : (g + 1) * QB]
        pack[:, g, :, QB : 2 * QB] = kt[:, :, g * QB : (g + 1) * QB]
    pack[:, :, :, 2 * QB : PK] = vp

    mask = _make_mask().astype(bf16)
    in_maps = []
    for c in range(NCORES):
        sl = slice(c * HPC, (c + 1) * HPC)
        in_maps.append(
            {
                "pk": np.ascontiguousarray(pack[sl]),
                "mask": mask,
            }
        )

    nc = _get_compiled()
    res = run_bass_kernel_spmd(nc, in_maps, core_ids=list(range(NCORES)))
    LAST_RESULT = res

    # Gather: 8 x [HPC, 128, 16, 130] -> [B, S, H, D], normalizing by the
    # denominator column (out[..., 128] = sum of exp'd scores for that row).
    o4 = np.concatenate([r["out"] for r in res.results], axis=0).astype(np.float32)
    o = o4[..., :D] / o4[..., D : D + 1]
    # row (p, qi) -> global q = qi*128 + p
    o = o.transpose(0, 2, 1, 3).reshape(B * H, S, D)
    o = o.reshape(B, H, S, D).transpose(0, 2, 1, 3)
    return np.ascontiguousarray(o, dtype=np.float32)


# revision 25
# speedup vs baseline: 1.0221x; 1.0221x over previous
"""Causal multi-head attention (B=2, S=2048, H=16, D=128, fp32) on 8 trn2 NeuronCores.

Sharding: the 32 (batch, head) pairs are split 4-per-core (head-parallel — the
endpoint of the Ulysses all-to-all; with full inputs on host, realized as the
host-side scatter/gather). Causal work per head is identical, so cores are
perfectly load-balanced and need no cross-core communication.

Device kernel (per core, per head): flash-style attention, S^T score layout,
P-stationary PV with a fused denominator column.
  - Host packs Q^T[d,s], K^T[d,s] and V_ext[s,130] (col 128 = 1.0 denominator
    column, col 129 pad) into one tensor per (head, 512-wide q/k-group):
    [qt 512 | kt 512 | v 4x130] = 1544 cols x 128 partitions, so each input
    group is ONE ~0.4MB DMA with 3KB partition lines (near-peak HBM BW).
  - QK: S^T[sk,sq] = KT_tile^T @ QT per 512-wide q-block, diagonal k-tiles
    trimmed and ragged-packed in pairs; full k-tiles batched 2 per PSUM tile.
  - exp via ACT (fused 1/sqrt(D) scale) for most tiles; late blocks offload
    some full k-tile groups to DVE via a Schraudolph int16/bf16 bit-trick exp
    so the exp cost splits across two engines. DVE also applies the
    triangular mask on the two diagonal 128-bands (strided mul).
  - PV: for each 128-row q-chunk, stationary = P^T chunk [sk,128sq], moving =
    V_ext[sk,130] -> PSUM chunk [128sq,130] accumulates [O | L | pad] over
    k-tiles: the softmax denominator L costs one extra moving column instead
    of ones-matmuls + DVE reduction trees.
  - DVE copies PSUM->SBUF bf16, one DMA out per block; host divides O by L
    during the gather (normalization off-device).
  - Blocks run in diagonal wavefront order (wave = head + j) with a
    one-block software pipeline; input DMAs are issued in need order.
"""

import math
import sys

sys.path.insert(0, "/opt/trn_rl_repo")

import numpy as np

B, S, H, D = 2, 2048, 16, 128
NCORES = 8
HPC = (B * H) // NCORES  # heads per core = 4
QB = 512                 # q-block width
NQB = S // QB            # 4
KT128 = S // 128         # 16 k-tiles per head
SCALE = 1.0 / math.sqrt(D)
GRP = 2                  # full k-tiles exp'd per ACT instruction
VW = 130                 # V width: 128 d + 1 ones (L) + 1 pad
PK = QB + QB + 4 * VW    # packed group width: qt 512 | kt 512 | v 4*130
# Schraudolph bf16 exp offload (DVE): exp(x*SCALE) ~ bitcast_bf16(int16(
# x*(2^7*log2(e)*SCALE) + (127*2^7 - C))). C centers the sawtooth error
# (rms ~1.8%); applied only to full k-tiles of late blocks where softmax
# rows average >= 512 terms, so the output error contribution stays ~1%.
SCH_A = (128.0 / math.log(2.0)) * (1.0 / math.sqrt(D))
SCH_B = 127.0 * 128.0 - 7.33

_COMPILED = {}
LAST_RESULT = None


def _build_bass():
    from contextlib import ExitStack

    import concourse.tile as tile
    from concourse import bacc, mybir

    f32 = mybir.dt.float32
    bf16 = mybir.dt.bfloat16
    Exp = mybir.ActivationFunctionType.Exp

    nc = bacc.Bacc(
        "TRN2",
        target_bir_lowering=False,
        debug=False,
        enable_asserts=False,
        num_devices=NCORES,
    )
    # packed input: [head, j-group, partition, qt|kt|v]
    p_d = nc.dram_tensor("pk", [HPC, NQB, 128, PK], bf16, kind="ExternalInput").ap()
    mk_d = nc.dram_tensor("mask", [128, 128], bf16, kind="ExternalInput").ap()
    # out[h, p, qi, :] = [O | L | pad] for query row qi*128 + p
    o_d = nc.dram_tensor("out", [HPC, 128, KT128, VW], bf16, kind="ExternalOutput").ap()

    with tile.TileContext(nc) as tc, ExitStack() as ctx:
        const = ctx.enter_context(tc.tile_pool(name="const", bufs=1))
        ptd0_pool = ctx.enter_context(tc.tile_pool(name="ptd0", bufs=3))
        ptd1_pool = ctx.enter_context(tc.tile_pool(name="ptd1", bufs=3))
        ptf_pool = ctx.enter_context(tc.tile_pool(name="ptf", bufs=8))
        osb_pool = ctx.enter_context(tc.tile_pool(name="osb", bufs=8))
        ps_s = ctx.enter_context(tc.tile_pool(name="ps_s", bufs=2, space="PSUM"))
        ps_v = ctx.enter_context(tc.tile_pool(name="ps_v", bufs=2, space="PSUM"))

        # per-head packed input tile: [128, j-group, qt|kt|v]
        pk = [
            const.tile([128, NQB, PK], bf16, name=f"pk{h}", tag=f"pk{h}")
            for h in range(HPC)
        ]
        mk_sb = const.tile([128, 128], bf16)

        def load(hh, g0, g1, eng):
            eng.dma_start(
                pk[hh][:, g0 : g1 + 1, :],
                p_d[hh][g0 : g1 + 1].rearrange("n p c -> p n c"),
            )

        # Input DMA issue in need order (wave = head + j). Head 0's first
        # group is split so the first matmul (diagonal pair m2/m3, which only
        # needs the [256:512] halves) can start as early as possible. Late
        # heads' groups are merged into single ~1.2MB DMAs for peak BW.
        nc.sync.dma_start(pk[0][:, 0, 256:QB], p_d[0][0][:, 256:QB])
        nc.scalar.dma_start(pk[0][:, 0, QB + 256 : 2 * QB], p_d[0][0][:, QB + 256 : 2 * QB])
        nc.sync.dma_start(pk[0][:, 0, 0:256], p_d[0][0][:, 0:256])
        nc.scalar.dma_start(pk[0][:, 0, QB : QB + 256], p_d[0][0][:, QB : QB + 256])
        nc.scalar.dma_start(mk_sb[:], mk_d[:])
        nc.sync.dma_start(pk[0][:, 0, 2 * QB : PK], p_d[0][0][:, 2 * QB : PK])
        loads = [
            (1, 0, 0),  # (hh, g0, g1) in need order
            (0, 1, 1),
            (1, 1, 1),
            (2, 0, 0),
            (0, 2, 2),
            (1, 2, 2),
            (2, 1, 1),
            (3, 0, 0),
            (0, 3, 3),
            (2, 2, 3),
            (1, 3, 3),
            (3, 1, 3),
        ]
        for n, (hh, g0, g1) in enumerate(loads):
            load(hh, g0, g1, nc.sync)

        def qt_ap(hh, j, w0):
            # columns [j*QB + w0, (j+1)*QB) of head hh's Q^T
            return pk[hh][:, j, w0:QB]

        def kt_ap(hh, ki):
            return pk[hh][:, ki // 4, QB + (ki % 4) * 128 : QB + (ki % 4 + 1) * 128]

        def v_ap(hh, ki):
            base = 2 * QB + (ki % 4) * VW
            return pk[hh][:, ki // 4, base : base + VW]

        # Diagonal k-tile m of a block: trimmed to width 512-128m, packed as
        # pair0 = {m0 @ 0 (512), m1 @ 512 (384)} in a [128,1024] tile and
        # pair1 = {m2 @ 0 (256), m3 @ 256 (128)} in a [128,512] tile.
        DIAG_OFF = {0: 0, 1: 512, 2: 0, 3: 256}

        def phase1(hh, j):
            """Allocate block tiles; return (state, generator). The generator
            emits QK matmuls + exp + mask one PSUM-group at a time, so the
            driver can interleave the previous block's PV chunks between
            groups (keeping PE fed while ACT drains the exp backlog)."""
            ptd0 = ptd0_pool.tile([128, 1024], bf16, tag="ptd0", name="ptd0")
            ptd1 = ptd1_pool.tile([128, 512], bf16, tag="ptd1", name="ptd1")
            nf = (4 * j + GRP - 1) // GRP
            ptfs = [
                ptf_pool.tile([128, GRP * QB], bf16, tag="ptf", name="ptf")
                for _ in range(nf)
            ]

            def gen():
                # j=0 emits pair1 (m2/m3) first: its inputs are the second
                # half of the first group, which lands first at startup.
                pair_order = (
                    ((1, ptd1), (0, ptd0)) if j == 0 else ((0, ptd0), (1, ptd1))
                )
                for pair, pt in pair_order:
                    ms = (0, 1) if pair == 0 else (2, 3)
                    tot = 896 if pair == 0 else 384
                    s_ps = ps_s.tile([128, GRP * QB], f32, tag="s", name="s_ps")
                    for m in ms:
                        w = QB - 128 * m
                        nc.tensor.matmul(
                            s_ps[:, DIAG_OFF[m] : DIAG_OFF[m] + w],
                            kt_ap(hh, 4 * j + m),
                            qt_ap(hh, j, 128 * m),
                            start=True,
                            stop=True,
                        )
                    nc.scalar.activation(
                        pt[:, 0:tot], s_ps[:, 0:tot], Exp, scale=SCALE
                    )
                    # Both 128-wide triangular bands of this pair sit at
                    # offsets {0, half}: mask with one strided DVE op.
                    half = 512 if pair == 0 else 256
                    ptv = pt[:, 0 : 2 * half].rearrange(
                        "p (a b) -> p a b", a=2, b=half
                    )[:, :, 0:128]
                    mkv = mk_sb[:].unsqueeze(1).broadcast_to([128, 2, 128])
                    nc.vector.tensor_mul(ptv, ptv, mkv)
                    yield
                # Full k-tiles, exp'd GRP at a time. Late blocks offload some
                # groups to DVE (Schraudolph) so exp splits across engines.
                for gi, g0 in enumerate(range(0, 4 * j, GRP)):
                    grp = min(GRP, 4 * j - g0)
                    s_ps = ps_s.tile([128, GRP * QB], f32, tag="s", name="s_ps")
                    ptf = ptfs[gi]
                    for idx in range(grp):
                        nc.tensor.matmul(
                            s_ps[:, idx * QB : (idx + 1) * QB],
                            kt_ap(hh, g0 + idx),
                            qt_ap(hh, j, 0),
                            start=True,
                            stop=True,
                        )
                    offload = (
                        (j == 1 and g0 == 2)
                        or (j == 2 and g0 == 6)
                        or (j == 3 and g0 >= 6)
                    )
                    if offload:
                        nc.vector.tensor_scalar(
                            ptf[:, 0 : grp * QB].bitcast(mybir.dt.int16),
                            s_ps[:, 0 : grp * QB],
                            SCH_A,
                            SCH_B,
                            mybir.AluOpType.mult,
                            mybir.AluOpType.add,
                        )
                    else:
                        nc.scalar.activation(
                            ptf[:, 0 : grp * QB],
                            s_ps[:, 0 : grp * QB],
                            Exp,
                            scale=SCALE,
                        )
                    yield

            return (hh, j, ptd0, ptd1, ptfs), gen()

        def pt_chunk(ptd0, ptd1, ptfs, j, ki, c):
            """Stationary AP: columns [c*128,(c+1)*128) of block j's P^T tile ki."""
            if ki < 4 * j:
                gi, idx = ki // GRP, ki % GRP
                base = idx * QB + c * 128
                return ptfs[gi][:, base : base + 128]
            m = ki - 4 * j
            pt = ptd0 if m < 2 else ptd1
            base = DIAG_OFF[m] + (c - m) * 128
            return pt[:, base : base + 128]

        def phase2(st):
            """Generator: PV accumulation [O | L | pad] one 128-row q-chunk
            per step, then copy PSUM->SBUF and DMA out."""
            hh, j, ptd0, ptd1, ptfs = st
            pv = [
                ps_v.tile([128, 2, VW], f32, tag="pv01", name="pv01"),
                ps_v.tile([128, 2, VW], f32, tag="pv23", name="pv23"),
            ]
            # Chunk-major: each PSUM accumulation group is contiguous (two
            # chunks share a bank; interleaving groups would corrupt the
            # bank-wide has_written bits).
            for c in range(4):
                dest = pv[c // 2][:, c % 2, :]
                last = 4 * j + c
                for ki in range(last + 1):
                    if ki >= 4 * j and (ki - 4 * j) > c:
                        continue
                    nc.tensor.matmul(
                        dest,
                        pt_chunk(ptd0, ptd1, ptfs, j, ki, c),
                        v_ap(hh, ki),
                        start=(ki == 0),
                        stop=(ki == last),
                    )
                yield
            osb = osb_pool.tile([128, 4, VW], bf16, tag="osb", name="osb")
            nc.vector.tensor_copy(osb[:, 0:2, :], pv[0][:])
            nc.vector.tensor_copy(osb[:, 2:4, :], pv[1][:])
            nc.sync.dma_start(o_d[hh][:, 4 * j : 4 * j + 4, :], osb[:])

        # One-block software pipeline with interleaved emission: the QK/exp
        # groups of block N+1 alternate with the PV chunks of block N in the
        # PE instruction stream, so whenever a QK group is gated on ACT
        # draining a PSUM tile, ready PV work sits right behind it.
        # Blocks run in diagonal wavefront order (wave s = head + j): early
        # waves touch few heads' inputs, so compute ramps while DMA streams
        # the rest, and consecutive blocks stay similar in shape.
        order = sorted(
            ((hh, j) for hh in range(HPC) for j in range(NQB)),
            key=lambda t: (t[0] + t[1], t[0]),
        )

        def drive(g1, g2):
            for _ in g1:
                pass
            for _ in g2:
                pass

        prev = None
        for hh, j in order:
            st, g1 = phase1(hh, j)
            drive(g1, phase2(prev) if prev is not None else iter(()))
            prev = st
        drive(iter(()), phase2(prev))

    nc.compile()
    return nc


def _get_compiled():
    if "nc" not in _COMPILED:
        _COMPILED["nc"] = _build_bass()
    return _COMPILED["nc"]


def _make_mask():
    k = np.arange(128, dtype=np.int64)[:, None]
    t = np.arange(128, dtype=np.int64)[None, :]
    return (t >= k).astype(np.float32)


def kernel(query, key, value):
    global LAST_RESULT
    from concourse.bass_utils import run_bass_kernel_spmd

    import ml_dtypes

    bf16 = ml_dtypes.bfloat16

    # [B, S, H, D] -> [B*H, S, D]
    q = np.asarray(query, dtype=np.float32).transpose(0, 2, 1, 3).reshape(B * H, S, D)
    k = np.asarray(key, dtype=np.float32).transpose(0, 2, 1, 3).reshape(B * H, S, D)
    v = np.asarray(value, dtype=np.float32).transpose(0, 2, 1, 3).reshape(B * H, S, D)

    qt = q.transpose(0, 2, 1).astype(bf16)  # [BH, D, S]
    kt = k.transpose(0, 2, 1).astype(bf16)

    v_ext = np.zeros((B * H, S, VW), dtype=bf16)
    v_ext[:, :, :D] = v.astype(bf16)
    v_ext[:, :, D] = bf16(1.0)
    # v packed per j-group: [BH, NQB, 128, 4*VW], row (n*128+p) -> [p, n*VW+d]
    vp = (
        v_ext.reshape(B * H, NQB, 4, 128, VW)
        .transpose(0, 1, 3, 2, 4)
        .reshape(B * H, NQB, 128, 4 * VW)
    )

    pack = np.empty((B * H, NQB, 128, PK), dtype=bf16)
    for g in range(NQB):
        pack[:, g, :, 0:QB] = qt[:, :, g * QB : (g + 1) * QB]
        pack[:, g, :, QB : 2 * QB] = kt[:, :, g * QB : (g + 1) * QB]
    pack[:, :, :, 2 * QB : PK] = vp

    mask = _make_mask().astype(bf16)
    in_maps = []
    for c in range(NCORES):
        sl = slice(c * HPC, (c + 1) * HPC)
        in_maps.append(
            {
                "pk": np.ascontiguousarray(pack[sl]),
                "mask": mask,
            }
        )

    nc = _get_compiled()
    res = run_bass_kernel_spmd(nc, in_maps, core_ids=list(range(NCORES)))
    LAST_RESULT = res

    # Gather: 8 x [HPC, 128, 16, 130] -> [B, S, H, D], normalizing by the
    # denominator column (out[..., 128] = sum of exp'd scores for that row).
    o4 = np.concatenate([r["out"] for r in res.results], axis=0).astype(np.float32)
    o = o4[..., :D] / o4[..., D : D + 1]
    # row (p, qi) -> global q = qi*128 + p
    o = o.transpose(0, 2, 1, 3).reshape(B * H, S, D)
    o = o.reshape(B, H, S, D).transpose(0, 2, 1, 3)
    return np.ascontiguousarray(o, dtype=np.float32)


# revision 26
# speedup vs baseline: 1.0366x; 1.0142x over previous
"""Causal multi-head attention (B=2, S=2048, H=16, D=128, fp32) on 8 trn2 NeuronCores.

Sharding: the 32 (batch, head) pairs are split 4-per-core (head-parallel — the
endpoint of the Ulysses all-to-all; with full inputs on host, realized as the
host-side scatter/gather). Causal work per head is identical, so cores are
perfectly load-balanced and need no cross-core communication.

Device kernel (per core, per head): flash-style attention, S^T score layout,
P-stationary PV with a fused denominator column.
  - Host packs Q^T[d,s], K^T[d,s] and V_ext[s,130] (col 128 = 1.0 denominator
    column, col 129 pad) into one tensor per (head, 512-wide q/k-group):
    [qt 512 | kt 512 | v 4x130] = 1544 cols x 128 partitions, so each input
    group is ONE ~0.4MB DMA with 3KB partition lines (near-peak HBM BW).
  - QK: S^T[sk,sq] = KT_tile^T @ QT per 512-wide q-block, diagonal k-tiles
    trimmed and ragged-packed in pairs; full k-tiles batched 2 per PSUM tile.
  - exp via ACT (fused 1/sqrt(D) scale) for most tiles; late blocks offload
    some full k-tile groups to DVE via a Schraudolph int16/bf16 bit-trick exp
    so the exp cost splits across two engines. DVE also applies the
    triangular mask on the two diagonal 128-bands (strided mul).
  - PV: for each 128-row q-chunk, stationary = P^T chunk [sk,128sq], moving =
    V_ext[sk,130] -> PSUM chunk [128sq,130] accumulates [O | L | pad] over
    k-tiles: the softmax denominator L costs one extra moving column instead
    of ones-matmuls + DVE reduction trees.
  - DVE copies PSUM->SBUF bf16, one DMA out per block; host divides O by L
    during the gather (normalization off-device).
  - Blocks run in diagonal wavefront order (wave = head + j) with a
    one-block software pipeline; input DMAs are issued in need order.
"""

import math
import sys

sys.path.insert(0, "/opt/trn_rl_repo")

import numpy as np

B, S, H, D = 2, 2048, 16, 128
NCORES = 8
HPC = (B * H) // NCORES  # heads per core = 4
QB = 512                 # q-block width
NQB = S // QB            # 4
KT128 = S // 128         # 16 k-tiles per head
SCALE = 1.0 / math.sqrt(D)
GRP = 2                  # full k-tiles exp'd per ACT instruction
VW = 130                 # V width: 128 d + 1 ones (L) + 1 pad
PK = QB + QB + 4 * VW    # packed group width: qt 512 | kt 512 | v 4*130
# Schraudolph bf16 exp offload (DVE): exp(x*SCALE) ~ bitcast_bf16(int16(
# x*(2^7*log2(e)*SCALE) + (127*2^7 - C))). C centers the sawtooth error
# (rms ~1.8%); applied only to full k-tiles of late blocks where softmax
# rows average >= 512 terms, so the output error contribution stays ~1%.
SCH_A = (128.0 / math.log(2.0)) * (1.0 / math.sqrt(D))
SCH_B = 127.0 * 128.0 - 7.33

_COMPILED = {}
LAST_RESULT = None


def _build_bass():
    from contextlib import ExitStack

    import concourse.tile as tile
    from concourse import bacc, mybir

    f32 = mybir.dt.float32
    bf16 = mybir.dt.bfloat16
    Exp = mybir.ActivationFunctionType.Exp

    nc = bacc.Bacc(
        "TRN2",
        target_bir_lowering=False,
        debug=False,
        enable_asserts=False,
        num_devices=NCORES,
    )
    # packed input: [head, j-group, partition, qt|kt|v]
    p_d = nc.dram_tensor("pk", [HPC, NQB, 128, PK], bf16, kind="ExternalInput").ap()
    mk_d = nc.dram_tensor("mask", [128, 128], bf16, kind="ExternalInput").ap()
    # out[h, p, qi, :] = [O | L | pad] for query row qi*128 + p
    o_d = nc.dram_tensor("out", [HPC, 128, KT128, VW], bf16, kind="ExternalOutput").ap()

    with tile.TileContext(nc) as tc, ExitStack() as ctx:
        const = ctx.enter_context(tc.tile_pool(name="const", bufs=1))
        ptd0_pool = ctx.enter_context(tc.tile_pool(name="ptd0", bufs=4))
        ptd1_pool = ctx.enter_context(tc.tile_pool(name="ptd1", bufs=4))
        ptf_pool = ctx.enter_context(tc.tile_pool(name="ptf", bufs=10))
        osb_pool = ctx.enter_context(tc.tile_pool(name="osb", bufs=8))
        ps_s = ctx.enter_context(tc.tile_pool(name="ps_s", bufs=2, space="PSUM"))
        ps_v = ctx.enter_context(tc.tile_pool(name="ps_v", bufs=2, space="PSUM"))

        # per-head packed input tile: [128, j-group, qt|kt|v]
        pk = [
            const.tile([128, NQB, PK], bf16, name=f"pk{h}", tag=f"pk{h}")
            for h in range(HPC)
        ]
        mk_sb = const.tile([128, 128], bf16)

        def load(hh, g0, g1, eng):
            eng.dma_start(
                pk[hh][:, g0 : g1 + 1, :],
                p_d[hh][g0 : g1 + 1].rearrange("n p c -> p n c"),
            )

        # Input DMA issue in need order (wave = head + j). Head 0's first
        # group is split so the first matmul (diagonal pair m2/m3, which only
        # needs the [256:512] halves) can start as early as possible. Late
        # heads' groups are merged into single ~1.2MB DMAs for peak BW.
        nc.sync.dma_start(pk[0][:, 0, 256:QB], p_d[0][0][:, 256:QB])
        nc.scalar.dma_start(pk[0][:, 0, QB + 256 : 2 * QB], p_d[0][0][:, QB + 256 : 2 * QB])
        nc.sync.dma_start(pk[0][:, 0, 0:256], p_d[0][0][:, 0:256])
        nc.scalar.dma_start(pk[0][:, 0, QB : QB + 256], p_d[0][0][:, QB : QB + 256])
        nc.scalar.dma_start(mk_sb[:], mk_d[:])
        nc.sync.dma_start(pk[0][:, 0, 2 * QB : PK], p_d[0][0][:, 2 * QB : PK])
        loads = [
            (1, 0, 0),  # (hh, g0, g1) in need order
            (0, 1, 1),
            (1, 1, 1),
            (2, 0, 0),
            (0, 2, 2),
            (1, 2, 2),
            (2, 1, 1),
            (3, 0, 0),
            (0, 3, 3),
            (2, 2, 3),
            (1, 3, 3),
            (3, 1, 3),
        ]
        for n, (hh, g0, g1) in enumerate(loads):
            load(hh, g0, g1, nc.sync)

        def qt_ap(hh, j, w0):
            # columns [j*QB + w0, (j+1)*QB) of head hh's Q^T
            return pk[hh][:, j, w0:QB]

        def kt_ap(hh, ki):
            return pk[hh][:, ki // 4, QB + (ki % 4) * 128 : QB + (ki % 4 + 1) * 128]

        def v_ap(hh, ki):
            base = 2 * QB + (ki % 4) * VW
            return pk[hh][:, ki // 4, base : base + VW]

        # Diagonal k-tile m of a block: trimmed to width 512-128m, packed as
        # pair0 = {m0 @ 0 (512), m1 @ 512 (384)} in a [128,1024] tile and
        # pair1 = {m2 @ 0 (256), m3 @ 256 (128)} in a [128,512] tile.
        DIAG_OFF = {0: 0, 1: 512, 2: 0, 3: 256}

        def phase1(hh, j):
            """Allocate block tiles; return (state, generator). The generator
            emits QK matmuls + exp + mask one PSUM-group at a time, so the
            driver can interleave the previous block's PV chunks between
            groups (keeping PE fed while ACT drains the exp backlog)."""
            ptd0 = ptd0_pool.tile([128, 1024], bf16, tag="ptd0", name="ptd0")
            ptd1 = ptd1_pool.tile([128, 512], bf16, tag="ptd1", name="ptd1")
            nf = (4 * j + GRP - 1) // GRP
            ptfs = [
                ptf_pool.tile([128, GRP * QB], bf16, tag="ptf", name="ptf")
                for _ in range(nf)
            ]

            def gen():
                # j=0 emits pair1 (m2/m3) first: its inputs are the second
                # half of the first group, which lands first at startup.
                pair_order = (
                    ((1, ptd1), (0, ptd0)) if j == 0 else ((0, ptd0), (1, ptd1))
                )
                for pair, pt in pair_order:
                    ms = (0, 1) if pair == 0 else (2, 3)
                    tot = 896 if pair == 0 else 384
                    s_ps = ps_s.tile([128, GRP * QB], f32, tag="s", name="s_ps")
                    for m in ms:
                        w = QB - 128 * m
                        nc.tensor.matmul(
                            s_ps[:, DIAG_OFF[m] : DIAG_OFF[m] + w],
                            kt_ap(hh, 4 * j + m),
                            qt_ap(hh, j, 128 * m),
                            start=True,
                            stop=True,
                        )
                    nc.scalar.activation(
                        pt[:, 0:tot], s_ps[:, 0:tot], Exp, scale=SCALE
                    )
                    # Both 128-wide triangular bands of this pair sit at
                    # offsets {0, half}: mask with one strided DVE op.
                    half = 512 if pair == 0 else 256
                    ptv = pt[:, 0 : 2 * half].rearrange(
                        "p (a b) -> p a b", a=2, b=half
                    )[:, :, 0:128]
                    mkv = mk_sb[:].unsqueeze(1).broadcast_to([128, 2, 128])
                    nc.vector.tensor_mul(ptv, ptv, mkv)
                    yield
                # Full k-tiles, exp'd GRP at a time. Late blocks offload some
                # groups to DVE (Schraudolph) so exp splits across engines.
                for gi, g0 in enumerate(range(0, 4 * j, GRP)):
                    grp = min(GRP, 4 * j - g0)
                    s_ps = ps_s.tile([128, GRP * QB], f32, tag="s", name="s_ps")
                    ptf = ptfs[gi]
                    for idx in range(grp):
                        nc.tensor.matmul(
                            s_ps[:, idx * QB : (idx + 1) * QB],
                            kt_ap(hh, g0 + idx),
                            qt_ap(hh, j, 0),
                            start=True,
                            stop=True,
                        )
                    offload = (
                        (j == 1 and g0 == 2)
                        or (j == 2 and g0 == 4)
                        or (j == 3 and g0 in (2, 6, 10))
                    )
                    if offload:
                        nc.vector.tensor_scalar(
                            ptf[:, 0 : grp * QB].bitcast(mybir.dt.int16),
                            s_ps[:, 0 : grp * QB],
                            SCH_A,
                            SCH_B,
                            mybir.AluOpType.mult,
                            mybir.AluOpType.add,
                        )
                    else:
                        nc.scalar.activation(
                            ptf[:, 0 : grp * QB],
                            s_ps[:, 0 : grp * QB],
                            Exp,
                            scale=SCALE,
                        )
                    yield

            return (hh, j, ptd0, ptd1, ptfs), gen()

        def pt_chunk(ptd0, ptd1, ptfs, j, ki, c):
            """Stationary AP: columns [c*128,(c+1)*128) of block j's P^T tile ki."""
            if ki < 4 * j:
                gi, idx = ki // GRP, ki % GRP
                base = idx * QB + c * 128
                return ptfs[gi][:, base : base + 128]
            m = ki - 4 * j
            pt = ptd0 if m < 2 else ptd1
            base = DIAG_OFF[m] + (c - m) * 128
            return pt[:, base : base + 128]

        def phase2(st):
            """Generator: PV accumulation [O | L | pad] one 128-row q-chunk
            per step, then copy PSUM->SBUF and DMA out."""
            hh, j, ptd0, ptd1, ptfs = st
            pv = [
                ps_v.tile([128, 2, VW], f32, tag="pv01", name="pv01"),
                ps_v.tile([128, 2, VW], f32, tag="pv23", name="pv23"),
            ]
            # Chunk-major: each PSUM accumulation group is contiguous (two
            # chunks share a bank; interleaving groups would corrupt the
            # bank-wide has_written bits).
            for c in range(4):
                dest = pv[c // 2][:, c % 2, :]
                last = 4 * j + c
                for ki in range(last + 1):
                    if ki >= 4 * j and (ki - 4 * j) > c:
                        continue
                    nc.tensor.matmul(
                        dest,
                        pt_chunk(ptd0, ptd1, ptfs, j, ki, c),
                        v_ap(hh, ki),
                        start=(ki == 0),
                        stop=(ki == last),
                    )
                yield
            osb = osb_pool.tile([128, 4, VW], bf16, tag="osb", name="osb")
            nc.vector.tensor_copy(osb[:, 0:2, :], pv[0][:])
            nc.vector.tensor_copy(osb[:, 2:4, :], pv[1][:])
            nc.sync.dma_start(o_d[hh][:, 4 * j : 4 * j + 4, :], osb[:])

        # One-block software pipeline with interleaved emission: the QK/exp
        # groups of block N+1 alternate with the PV chunks of block N in the
        # PE instruction stream, so whenever a QK group is gated on ACT
        # draining a PSUM tile, ready PV work sits right behind it.
        # Blocks run in diagonal wavefront order (wave s = head + j): early
        # waves touch few heads' inputs, so compute ramps while DMA streams
        # the rest, and consecutive blocks stay similar in shape.
        order = sorted(
            ((hh, j) for hh in range(HPC) for j in range(NQB)),
            key=lambda t: (t[0] + t[1], t[0]),
        )

        def drive(g1, g2):
            for _ in g1:
                pass
            for _ in g2:
                pass

        prev = None
        for hh, j in order:
            st, g1 = phase1(hh, j)
            drive(g1, phase2(prev) if prev is not None else iter(()))
            prev = st
        drive(iter(()), phase2(prev))

    nc.compile()
    return nc


def _get_compiled():
    if "nc" not in _COMPILED:
        _COMPILED["nc"] = _build_bass()
    return _COMPILED["nc"]


def _make_mask():
    k = np.arange(128, dtype=np.int64)[:, None]
    t = np.arange(128, dtype=np.int64)[None, :]
    return (t >= k).astype(np.float32)


def kernel(query, key, value):
    global LAST_RESULT
    from concourse.bass_utils import run_bass_kernel_spmd

    import ml_dtypes

    bf16 = ml_dtypes.bfloat16

    # [B, S, H, D] -> [B*H, S, D]
    q = np.asarray(query, dtype=np.float32).transpose(0, 2, 1, 3).reshape(B * H, S, D)
    k = np.asarray(key, dtype=np.float32).transpose(0, 2, 1, 3).reshape(B * H, S, D)
    v = np.asarray(value, dtype=np.float32).transpose(0, 2, 1, 3).reshape(B * H, S, D)

    qt = q.transpose(0, 2, 1).astype(bf16)  # [BH, D, S]
    kt = k.transpose(0, 2, 1).astype(bf16)

    v_ext = np.zeros((B * H, S, VW), dtype=bf16)
    v_ext[:, :, :D] = v.astype(bf16)
    v_ext[:, :, D] = bf16(1.0)
    # v packed per j-group: [BH, NQB, 128, 4*VW], row (n*128+p) -> [p, n*VW+d]
    vp = (
        v_ext.reshape(B * H, NQB, 4, 128, VW)
        .transpose(0, 1, 3, 2, 4)
        .reshape(B * H, NQB, 128, 4 * VW)
    )

    pack = np.empty((B * H, NQB, 128, PK), dtype=bf16)
    for g in range(NQB):
        pack[:, g, :, 0:QB] = qt[:, :, g * QB : (g + 1) * QB]
        pack[:, g, :, QB : 2 * QB] = kt[:, :, g * QB : (g + 1) * QB]
    pack[:, :, :, 2 * QB : PK] = vp

    mask = _make_mask().astype(bf16)
    in_maps = []
    for c in range(NCORES):
        sl = slice(c * HPC, (c + 1) * HPC)
        in_maps.append(
            {
                "pk": np.ascontiguousarray(pack[sl]),
                "mask": mask,
            }
        )

    nc = _get_compiled()
    res = run_bass_kernel_spmd(nc, in_maps, core_ids=list(range(NCORES)))
    LAST_RESULT = res

    # Gather: 8 x [HPC, 128, 16, 130] -> [B, S, H, D], normalizing by the
    # denominator column (out[..., 128] = sum of exp'd scores for that row).
    o4 = np.concatenate([r["out"] for r in res.results], axis=0).astype(np.float32)
    o = o4[..., :D] / o4[..., D : D + 1]
    # row (p, qi) -> global q = qi*128 + p
    o = o.transpose(0, 2, 1, 3).reshape(B * H, S, D)
    o = o.reshape(B, H, S, D).transpose(0, 2, 1, 3)
    return np.ascontiguousarray(o, dtype=np.float32)
